# revision 1
# baseline (speedup 1.0000x reference)
"""EquivariantGNN layer on 8 Trainium2 NeuronCores.

Data-parallel over the 256 graphs (32 graphs/core, processed as 16
pairs packed on 128 partitions). Per pair, the N^2 edge work lives in a
[128, 4096] "transposed" layout: partitions = (graph, channel), free =
(j, i) with i innermost.

Algebraic restructuring vs the reference:
  - edge MLP layer 1 is separable: e_in @ W1 = HA[i] + HB[j] + w1r*d2
    -> built by PE matmuls straight into PSUM (identity-pattern rhs),
    b1 folded into the relu bias.
  - coord MLP layer 1 is fused into edge layer 2: t = r1 @ (W2@C1),
    so m_ij is never materialized; m_i = (sum_j r1) @ W2.
  - pos aggregation via sum_j w_ij * [pos_j|1] = one K=128 matmul after
    un-flattening w into a block-diagonal [128,128] tile by DMA.
"""

import sys

sys.path.insert(0, "/opt/trn_rl_repo")

import numpy as np

import concourse.bass as bass
import concourse.tile as tile
from concourse import bacc, mybir
from concourse.bass_utils import run_bass_kernel_spmd

N = 64          # nodes per graph
H = 64          # hidden
D = 6           # in/out feature dim
B, S = 8, 32
G = B * S       # 256 graphs
NCORES = 8
GPC = G // NCORES   # 32 graphs per core
PAIRS = GPC // 2    # 16 pairs
NSQ = N * N         # 4096
CH = 512            # free-dim chunk (one PSUM bank of f32)
NCH = NSQ // CH     # 8 chunks per pair
JB = CH // N        # 8 j-values per chunk

F32 = mybir.dt.float32
F32R = mybir.dt.float32r
BF16 = mybir.dt.bfloat16

# ---- perf knobs ----
MM_F32R = False     # bitcast f32 matmul operands to float32r (4x PE rate)
R1_BF16 = False     # r1/u intermediates in bf16 (faster DVE, ~4e-3 rounding)

R1DT = BF16 if R1_BF16 else F32
AF = mybir.ActivationFunctionType
OP = mybir.AluOpType


def _mm(ap):
    """Matmul-operand view: float32 -> float32r when enabled."""
    if MM_F32R and ap.dtype == F32:
        return ap.bitcast(F32R)
    return ap


def _mmdt(dt):
    if MM_F32R and dt == F32:
        return F32R
    return dt


def build_program():
    nc = bacc.Bacc("TRN2", target_bir_lowering=False, debug=False)
    dt_r1 = R1DT

    # ---------------- DRAM I/O ----------------
    xD = nc.dram_tensor("x", [GPC, N, D], F32, kind="ExternalInput").ap()
    outD = nc.dram_tensor("out", [GPC, N, D], F32, kind="ExternalOutput").ap()

    def cin(name, shape, dt=F32):
        return nc.dram_tensor(name, list(shape), dt, kind="ExternalInput").ap()

    embbdD = cin("embbd", [38, 2 * H])              # padded blockdiag emb_w
    I128D = cin("I128", [2 * H, 2 * H])             # identity for transpose
    b1nrD = cin("b1nr", [1, 2 * H])                 # node_b1 dup as row
    fb3rD = cin("fb3r", [1, 3])                     # final_b[3:] row
    onesrD = cin("onesr", [1, 2 * H])               # ones row
    embbD = cin("embb", [2 * H, 1])                 # emb_b dup
    AbdD = cin("Abd", [2 * H, 2 * H])               # blockdiag edge_w1[:H]
    BbdD = cin("Bbd", [2 * H, 2 * H])               # blockdiag edge_w1[H:2H]
    w1rmD = cin("w1rm", [2, 2 * H])                 # masked w1r rows
    b1cD = cin("b1c", [2 * H, 1])                   # edge_b1 dup
    TbigD = cin("Tbig", [N, NSQ])                   # delta_{i,i'} pattern
    Tbig2D = cin("Tbig2", [N, NSQ])                 # delta_{j,j'} pattern
    maskbdD = cin("maskbd", [2, 2 * H])             # graph masks
    W2C1bdD = cin("W2C1bd", [2 * H, 2 * H], dt_r1)  # blockdiag edge_w2@coord_w1
    b2c1cD = cin("b2c1c", [2 * H, 1])               # (b2@C1 + c1b) dup
    c2scD = cin("c2sc", [2 * H, 16 * NCH], dt_r1)   # shifted coord_w2 cols
    W2bdD = cin("W2bd", [2 * H, 2 * H], dt_r1)      # blockdiag edge_w2
    b2x64D = cin("b2x64", [2 * H, 1])               # 64*edge_b2 dup
    W1nh2D = cin("W1nh2", [2 * H, H])               # node_w1[:H] dup'd
    W1nm2D = cin("W1nm2", [2 * H, H])               # node_w1[H:] dup'd
    W2nbdD = cin("W2nbd", [2 * H, 2 * H])           # blockdiag node_w2
    b2ncD = cin("b2nc", [2 * H, 1])                 # node_b2 dup
    Wf3D = cin("Wf3", [2 * H, 3])                   # final_w[:, 3:] dup'd

    from contextlib import ExitStack

    with tile.TileContext(nc) as tc:
        with ExitStack() as ctx:
            statics = ctx.enter_context(tc.tile_pool(name="statics", bufs=1))
            pers = ctx.enter_context(tc.tile_pool(name="pers", bufs=1))
            sb2 = ctx.enter_context(tc.tile_pool(name="sb2", bufs=2))
            big = ctx.enter_context(tc.tile_pool(name="big", bufs=2))
            zp = ctx.enter_context(tc.tile_pool(name="zp", bufs=2, space="PSUM"))
            tp = ctx.enter_context(tc.tile_pool(name="tp", bufs=2, space="PSUM"))
            sp = ctx.enter_context(tc.tile_pool(name="sp", bufs=2, space="PSUM"))
            dsc = ctx.enter_context(tc.tile_pool(name="dsc", bufs=2,
                                                 space="DRAM"))
            # ---- load constants into SBUF once ----
            def ld(apD, dt=None):
                t = statics.tile(list(apD.shape), dt or apD.dtype,
                                 tag=f"c_{apD.name}")
                nc.sync.dma_start(out=t[:], in_=apD)
                return t

            embbd = ld(embbdD)
            I128 = ld(I128D)
            b1nr = ld(b1nrD)
            fb3r = ld(fb3rD)
            onesr = ld(onesrD)
            embb = ld(embbD)
            Abd = ld(AbdD)
            Bbd = ld(BbdD)
            b1c = ld(b1cD)
            Tbig2 = ld(Tbig2D)
            maskbd = ld(maskbdD)
            W2C1bd = ld(W2C1bdD)
            b2c1c = ld(b2c1cD)
            c2sc = ld(c2scD)
            W2bd = ld(W2bdD)
            b2x64 = ld(b2x64D)
            W1nh2 = ld(W1nh2D)
            W1nm2 = ld(W1nm2D)
            W2nbd = ld(W2nbdD)
            b2nc = ld(b2ncD)
            Wf3 = ld(Wf3D)

            # persistent per-parity combo tiles
            cA_lhs = []  # [66, 128]: rows 0:64 HAstack (per pair), 64:66 w1r
            cA_rhs = []  # [66, 4096]: rows 0:64 Tbig static, 64:66 d2 flat
            wT = []      # [128, 128] blockdiag w target, zeroed once
            for par in range(2):
                lt = pers.tile([N + 2, 2 * H], F32, tag=f"cAl{par}")
                nc.sync.dma_start(out=lt[N:N + 2, :], in_=w1rmD)
                rt = pers.tile([N + 2, NSQ], F32, tag=f"cAr{par}")
                nc.sync.dma_start(out=rt[0:N, :], in_=TbigD)
                wt = pers.tile([2 * H, 2 * H], F32, tag=f"wT{par}")
                nc.vector.memset(wt[:], 0.0)
                cA_lhs.append(lt)
                cA_rhs.append(rt)
                wT.append(wt)

            for p in range(PAIRS):
                par = p % 2
                # ---- per-pair loads ----
                x_pair = sb2.tile([2 * N, D], F32)
                nc.gpsimd.dma_start(
                    out=x_pair[:],
                    in_=xD[2 * p:2 * p + 2, :, :].rearrange(
                        "g i d -> (g i) d"),
                )
                # x^T via PE transpose + DRAM bounce into padded per-graph
                # layout (graph 0 rows 0:6, graph 1 rows 32:38 -- PE-legal
                # partition bases)
                pxT = sp.tile([D, 2 * N], F32, tag="sp")
                nc.tensor.transpose(pxT[:], x_pair[:], I128[:])
                xT6 = sb2.tile([D, 2 * N], F32)
                nc.vector.tensor_copy(out=xT6[:], in_=pxT[:])
                xts = dsc.tile([D, 2 * N], F32, tag="xts")
                nc.gpsimd.dma_start(out=xts[:], in_=xT6[:])
                xT12p = sb2.tile([38, N], F32)
                nc.vector.memset(xT12p[:], 0.0)
                for gg in range(2):
                    nc.gpsimd.dma_start(out=xT12p[32 * gg:32 * gg + D, :],
                                        in_=xts[:, gg * N:(gg + 1) * N])

                # ---- embedding (transposed pair layout) ----
                ph = sp.tile([2 * H, N], F32, tag="sp")
                nc.tensor.matmul(ph[:], _mm(embbd[:]), _mm(xT12p[:]),
                                 start=True, stop=True)
                hT2 = sb2.tile([2 * H, N], F32)
                nc.vector.tensor_scalar_add(hT2[:], ph[:], embb[:, 0:1])

                # ---- HA/HB stacks ----
                pHA = sp.tile([N, 2 * H], F32, tag="sp")
                nc.tensor.matmul(pHA[:], _mm(hT2[:]), _mm(Abd[:]),
                                 start=True, stop=True)
                nc.vector.tensor_copy(out=cA_lhs[par][0:N, :], in_=pHA[:])
                pHB = sp.tile([N, 2 * H], F32, tag="sp")
                nc.tensor.matmul(pHB[:], _mm(hT2[:]), _mm(Bbd[:]),
                                 start=True, stop=True)
                HBst = sb2.tile([N, 2 * H], F32)
                nc.scalar.copy(out=HBst[:], in_=pHB[:])

                # ---- pairwise distances d2 ----
                pm2c = sb2.tile([38, N], F32)
                nc.vector.tensor_scalar_mul(pm2c[:], xT12p[:], -2.0)
                posTbd = sb2.tile([38, 2 * N], F32)
                nc.vector.memset(posTbd[:], 0.0)
                for gg in range(2):
                    nc.vector.tensor_copy(
                        out=posTbd[32 * gg:32 * gg + 3,
                                   gg * N:(gg + 1) * N],
                        in_=xT12p[32 * gg:32 * gg + 3, :])
                n2c = sb2.tile([2 * N, 1], F32)
                sq3 = sb2.tile([2 * N, 3], F32)
                nc.vector.scalar_tensor_tensor(
                    out=sq3[:], in0=x_pair[:, 0:3], scalar=1.0,
                    in1=x_pair[:, 0:3], op0=OP.mult, op1=OP.mult,
                    accum_out=n2c[:, 0:1])
                n2s = dsc.tile([2 * N, 1], F32, tag="n2s")
                nc.gpsimd.dma_start(out=n2s[:], in_=n2c[:])
                n2r = sb2.tile([2, N], F32)
                nc.gpsimd.dma_start(
                    out=n2r[:],
                    in_=n2s[:].rearrange("(g j) o -> g (j o)", g=2))
                pg = sp.tile([2 * H, N], F32, tag="sp")
                nc.tensor.matmul(pg[:], _mm(posTbd[:]), _mm(pm2c[:]),
                                 start=True, stop=False)
                nc.tensor.matmul(pg[:], _mm(maskbd[:]), _mm(n2r[:]),
                                 start=False, stop=True)
                d2 = sb2.tile([2 * N, N], F32)
                nc.vector.tensor_scalar_add(d2[:], pg[:], n2c[:, 0:1])
                # flatten [(g j), i] -> combo rows [g, (j i)] via DRAM
                # bounce; valid because d2 is symmetric
                d2s = dsc.tile([2 * N, N], F32, tag="d2s")
                nc.gpsimd.dma_start(out=d2s[:], in_=d2[:])
                nc.gpsimd.dma_start(
                    out=cA_rhs[par][N:N + 2, :],
                    in_=d2s[:].rearrange("(g j) i -> g (j i)", g=2))

                # ---- big phase: z1 -> r1 -> t -> u -> w ----
                r1 = big.tile([2 * H, NSQ], dt_r1, tag="r1")
                u = big.tile([2 * H, NSQ], dt_r1, tag="u")
                pw = sp.tile([2 * NCH, CH], F32, tag="sp")
                for cb in range(NCH):
                    pz = zp.tile([2 * H, CH], F32, tag="z")
                    nc.tensor.matmul(
                        pz[:], _mm(cA_lhs[par][:]),
                        _mm(cA_rhs[par][:, cb * CH:(cb + 1) * CH]),
                        start=True, stop=False)
                    nc.tensor.matmul(
                        pz[:], _mm(HBst[:]),
                        _mm(Tbig2[:, cb * CH:(cb + 1) * CH]),
                        start=False, stop=True)
                    r1_sl = r1[:, cb * CH:(cb + 1) * CH]
                    if cb % 2 == 0:
                        nc.scalar.activation(out=r1_sl, in_=pz[:], func=AF.Relu,
                                             bias=b1c[:, 0:1], scale=1.0)
                    else:
                        nc.vector.tensor_scalar(
                            out=r1_sl, in0=pz[:], scalar1=b1c[:, 0:1],
                            scalar2=0.0, op0=OP.add, op1=OP.max)
                    pt = tp.tile([2 * H, CH], F32, tag="t")
                    nc.tensor.matmul(pt[:], W2C1bd[:].bitcast(_mmdt(R1DT)),
                                     _mm(r1_sl), start=True, stop=True)
                    u_sl = u[:, cb * CH:(cb + 1) * CH]
                    if cb % 2 == 1:
                        nc.scalar.activation(out=u_sl, in_=pt[:], func=AF.Relu,
                                             bias=b2c1c[:, 0:1], scale=1.0)
                    else:
                        nc.vector.tensor_scalar(
                            out=u_sl, in0=pt[:], scalar1=b2c1c[:, 0:1],
                            scalar2=0.0, op0=OP.add, op1=OP.max)
                    nc.tensor.matmul(pw[:],
                                     c2sc[:, 16 * cb:16 * (cb + 1)]
                                     .bitcast(_mmdt(R1DT)), _mm(u_sl),
                                     start=(cb == 0), stop=(cb == NCH - 1),
                                     skip_group_check=True)

                # ---- w -> blockdiag via SBUF + DRAM bounce ----
                w16 = sb2.tile([2 * NCH, CH], F32)
                nc.scalar.copy(out=w16[:], in_=pw[:])
                ws = dsc.tile([2 * NCH, CH], F32, tag="ws")
                nc.gpsimd.dma_start(out=ws[:], in_=w16[:])
                for gg in range(2):
                    # DRAM-side gather: row 2cb+gg, chunk-local (jl, i)
                    nc.gpsimd.dma_start(
                        out=wT[par][gg * N:(gg + 1) * N,
                                    gg * N:(gg + 1) * N],
                        in_=ws[:].rearrange("(cb g) (jl i) -> g cb jl i",
                                            g=2, jl=JB)[gg],
                    )

                # ---- R = sum_j r1 (log-tree fold over outer j) ----
                tr = big.tile([2 * H, NSQ // 2], dt_r1, tag="tr")
                nc.vector.tensor_add(tr[:], r1[:, 0:NSQ // 2],
                                     r1[:, NSQ // 2:NSQ])
                wdt = NSQ // 4
                while wdt >= N:
                    nc.vector.tensor_add(tr[:, 0:wdt], tr[:, 0:wdt],
                                         tr[:, wdt:2 * wdt])
                    wdt //= 2
                # ---- m = R @ W2 + 64*b2 (T layout) ----
                pm = sp.tile([2 * H, N], F32, tag="sp")
                nc.tensor.matmul(pm[:], W2bd[:].bitcast(_mmdt(R1DT)),
                                 _mm(tr[:, 0:N]), start=True, stop=True)
                mTs = sb2.tile([2 * H, N], F32)
                nc.vector.tensor_scalar_add(mTs[:], pm[:], b2x64[:, 0:1])

                # ---- node MLP ----
                # init-mm seeds the full bank with the bias so the per-graph
                # matmuls can all accumulate (single start per bank region)
                pq = sp.tile([2 * H, N], F32, tag="sp")
                nc.tensor.matmul(pq[:], _mm(b1nr[:]), _mm(onesr[:, 0:N]),
                                 start=True, stop=False)
                for gg in range(2):
                    o = pq[gg * H:(gg + 1) * H, :]
                    sl = slice(gg * H, (gg + 1) * H)
                    nc.tensor.matmul(o, _mm(W1nh2[sl, :]), _mm(hT2[sl, :]),
                                     start=False, stop=False)
                    nc.tensor.matmul(o, _mm(W1nm2[sl, :]), _mm(mTs[sl, :]),
                                     start=False, stop=(gg == 1))
                qT = sb2.tile([2 * H, N], F32)
                nc.scalar.activation(out=qT[:], in_=pq[:], func=AF.Relu,
                                     bias=0.0, scale=1.0)
                pn2 = sp.tile([2 * H, N], F32, tag="sp")
                nc.tensor.matmul(pn2[:], _mm(W2nbd[:]), _mm(qT[:]),
                                 start=True, stop=True)
                hp = sb2.tile([2 * H, N], F32)
                nc.vector.scalar_tensor_tensor(
                    out=hp[:], in0=pn2[:], scalar=b2nc[:, 0:1], in1=hT2[:],
                    op0=OP.add, op1=OP.add)

                # ---- coordinate aggregation ----
                pa = sb2.tile([2 * N, 4], F32)
                nc.vector.tensor_copy(out=pa[:, 0:3], in_=x_pair[:, 0:3])
                nc.vector.memset(pa[:, 3:4], 1.0)
                pswp = sp.tile([2 * N, 4], F32, tag="sp")
                nc.tensor.matmul(pswp[:], _mm(wT[par][:]), _mm(pa[:]),
                                 start=True, stop=True)

                # ---- velocity head ----
                pv = sp.tile([2 * N, 3], F32, tag="sp")
                nc.tensor.matmul(pv[:], _mm(onesr[:]), _mm(fb3r[:]),
                                 start=True, stop=False)
                for gg in range(2):
                    sl = slice(gg * H, (gg + 1) * H)
                    nc.tensor.matmul(pv[gg * N:(gg + 1) * N, :],
                                     _mm(hp[sl, :]), _mm(Wf3[sl, :]),
                                     start=False, stop=(gg == 1))

                # ---- assemble output ----
                op_t = sb2.tile([2 * N, D], F32)
                tmp3 = sb2.tile([2 * N, 3], F32)
                nc.vector.scalar_tensor_tensor(
                    out=tmp3[:], in0=x_pair[:, 0:3], scalar=pswp[:, 3:4],
                    in1=pswp[:, 0:3], op0=OP.mult, op1=OP.subtract)
                nc.vector.scalar_tensor_tensor(
                    out=op_t[:, 0:3], in0=tmp3[:], scalar=1.0 / N,
                    in1=x_pair[:, 0:3], op0=OP.mult, op1=OP.add)
                nc.vector.tensor_add(op_t[:, 3:6], pv[:], x_pair[:, 3:6])
                nc.gpsimd.dma_start(
                    out=outD[2 * p:2 * p + 2, :, :].rearrange(
                        "g i d -> (g i) d"),
                    in_=op_t[:])

    nc.compile()
    return nc


def make_consts(emb_w, emb_b, edge_w1, edge_b1, edge_w2, edge_b2,
                node_w1, node_b1, node_w2, node_b2,
                coord_w1, coord_b1, coord_w2, final_w, final_b):
    f = np.float32
    E = np.asarray(emb_w, f)
    A = np.asarray(edge_w1[0:H], f)
    Bm = np.asarray(edge_w1[H:2 * H], f)
    w1r = np.asarray(edge_w1[2 * H], f)
    W2 = np.asarray(edge_w2, f)
    C1 = np.asarray(coord_w1, f)
    W2C1 = (W2 @ C1).astype(f)
    b2c1 = (np.asarray(edge_b2, f) @ C1 + np.asarray(coord_b1, f)).astype(f)
    c2 = np.asarray(coord_w2, f)[:, 0]

    def bd(M):
        out = np.zeros((2 * M.shape[0], 2 * M.shape[1]), f)
        out[:M.shape[0], :M.shape[1]] = M
        out[M.shape[0]:, M.shape[1]:] = M
        return out

    def dup(v):
        return np.tile(np.asarray(v, f), 2)[:, None].astype(f)

    w1rm = np.zeros((2, 2 * H), f)
    w1rm[0, 0:H] = w1r
    w1rm[1, H:2 * H] = w1r
    # c2sc: for chunk cb, lhsT = c2sc[:, 16cb:16cb+16]; column m of that
    # slice carries c2 masked to graph gg iff m == 2*cb+gg, so the 8
    # accumulating matmuls scatter chunk cb's w into psum rows 2cb:2cb+2.
    c2sc = np.zeros((2 * H, 16 * NCH), f)
    for cb in range(NCH):
        for gg in range(2):
            c2sc[gg * H:(gg + 1) * H, 16 * cb + 2 * cb + gg] = c2
    maskbd = np.zeros((2, 2 * H), f)
    maskbd[0, 0:N] = 1.0
    maskbd[1, N:2 * N] = 1.0
    r1dt = np.dtype("bfloat16") if R1_BF16 else f
    W1 = np.asarray(node_w1, f)
    embbd38 = np.zeros((38, 2 * H), f)
    embbd38[0:D, 0:H] = E
    embbd38[32:32 + D, H:2 * H] = E
    consts = {
        "embbd": embbd38,
        "I128": np.eye(2 * H, dtype=f),
        "b1nr": np.tile(np.asarray(node_b1, f), 2)[None, :],
        "fb3r": np.asarray(final_b, f)[None, 3:6],
        "onesr": np.ones((1, 2 * H), f),
        "embb": dup(emb_b),
        "Abd": bd(A),
        "Bbd": bd(Bm),
        "w1rm": w1rm,
        "b1c": dup(edge_b1),
        "Tbig": np.tile(np.eye(N, dtype=f), (1, N)),
        "Tbig2": np.kron(np.eye(N, dtype=f), np.ones((1, N), f)),
        "maskbd": maskbd,
        "W2C1bd": bd(W2C1).astype(r1dt),
        "b2c1c": dup(b2c1),
        "c2sc": c2sc.astype(r1dt),
        "W2bd": bd(W2).astype(r1dt),
        "b2x64": dup(np.asarray(edge_b2, f) * N),
        "W1nh2": np.concatenate([W1[0:H], W1[0:H]], 0),
        "W1nm2": np.concatenate([W1[H:2 * H], W1[H:2 * H]], 0),
        "W2nbd": bd(np.asarray(node_w2, f)),
        "b2nc": dup(node_b2),
        "Wf3": np.tile(np.asarray(final_w, f)[:, 3:6], (2, 1)),
    }
    return consts


_CACHE = {}


def _get_program():
    if "nc" not in _CACHE:
        _CACHE["nc"] = build_program()
    return _CACHE["nc"]


def kernel(x, emb_w, emb_b, edge_w1, edge_b1, edge_w2, edge_b2,
           node_w1, node_b1, node_w2, node_b2,
           coord_w1, coord_b1, coord_w2, final_w, final_b,
           _return_bass_results=False, _trace=False):
    nc = _get_program()
    consts = make_consts(emb_w, emb_b, edge_w1, edge_b1, edge_w2, edge_b2,
                         node_w1, node_b1, node_w2, node_b2,
                         coord_w1, coord_b1, coord_w2, final_w, final_b)
    xf = np.asarray(x, np.float32).reshape(G, N, D)
    in_maps = []
    for c in range(NCORES):
        m = dict(consts)
        m["x"] = np.ascontiguousarray(xf[c * GPC:(c + 1) * GPC])
        in_maps.append(m)
    res = run_bass_kernel_spmd(nc, in_maps, core_ids=list(range(NCORES)),
                               trace=_trace)
    out = np.concatenate([res.results[c]["out"] for c in range(NCORES)], 0)
    out = out.reshape(B, S, N, D).astype(np.float32)
    if _return_bass_results:
        return out, res
    return out



# revision 3
# speedup vs baseline: 17.3590x; 17.3590x over previous
"""EquivariantGNN layer on 8 Trainium2 NeuronCores.

Data-parallel over the 256 graphs (32 graphs/core, processed as 16
pairs packed on 128 partitions). Per pair, the N^2 edge work lives in a
[128, 4096] "transposed" layout: partitions = (graph, channel), free =
(j, i) with i innermost.

Algebraic restructuring vs the reference:
  - edge MLP layer 1 is separable: e_in @ W1 = HA[i] + HB[j] + w1r*d2
    -> built by PE matmuls straight into PSUM (identity-pattern rhs),
    b1 folded into the relu bias.
  - coord MLP layer 1 is fused into edge layer 2: t = r1 @ (W2@C1),
    so m_ij is never materialized; m_i = (sum_j r1) @ W2.
  - pos aggregation via sum_j w_ij * [pos_j|1] = one K=128 matmul after
    un-flattening w into a block-diagonal [128,128] tile by DMA.

Host/runtime restructuring vs the v0 session:
  - x and out cross the (slow, ~57ms RTT / ~25MB/s) axon tunnel in
    fp16, halving activation transfer bytes.
  - the jitted SPMD executable is cached across kernel() calls
    (rebuilding it cost ~0.6s/call in re-trace + BIR re-verify).
  - weight-derived constant tensors are uploaded once and kept
    device-resident across calls (keyed by a content digest); only x
    moves per call.
  - big-phase matmuls run as float32r (4x PE rate at N>=512) and the
    r1/u edge intermediates are bf16.
"""

import hashlib
import sys

sys.path.insert(0, "/opt/trn_rl_repo")

import numpy as np

import concourse.bass as bass  # noqa: F401  (registers AP machinery)
import concourse.tile as tile
from concourse import bacc, mybir

N = 64          # nodes per graph
H = 64          # hidden
D = 6           # in/out feature dim
B, S = 8, 32
G = B * S       # 256 graphs
NCORES = 8
GPC = G // NCORES   # 32 graphs per core
PAIRS = GPC // 2    # 16 pairs
NSQ = N * N         # 4096
CH = 512            # free-dim chunk (one PSUM bank of f32)
NCH = NSQ // CH     # 8 chunks per pair
JB = CH // N        # 8 j-values per chunk

F32 = mybir.dt.float32
F32R = mybir.dt.float32r
BF16 = mybir.dt.bfloat16
F16 = mybir.dt.float16

# ---- perf knobs ----
MM_F32R = False     # float32r needs explicit rounding chains; bf16 instead
Z_BF16 = True       # z-matmul operands (HA/HB/d2/patterns) in bf16 (1 cyc/row)
R1_BF16 = True      # r1/u intermediates in bf16 (faster DVE, ~4e-3 rounding)
IO_F16 = True       # x/out cross the axon tunnel as fp16

R1DT = BF16 if R1_BF16 else F32
ZDT = BF16 if Z_BF16 else F32
IODT = F16 if IO_F16 else F32
IODT_NP = np.float16 if IO_F16 else np.float32
AF = mybir.ActivationFunctionType
OP = mybir.AluOpType

WEIGHT_NAMES = [
    "emb_w", "emb_b", "edge_w1", "edge_b1", "edge_w2", "edge_b2",
    "node_w1", "node_b1", "node_w2", "node_b2",
    "coord_w1", "coord_b1", "coord_w2", "final_w", "final_b",
]


def _mm(ap):
    """Matmul-operand view: float32 -> float32r when enabled."""
    if MM_F32R and ap.dtype == F32:
        return ap.bitcast(F32R)
    return ap


def _mmdt(dt):
    if MM_F32R and dt == F32:
        return F32R
    return dt


def build_program():
    nc = bacc.Bacc("TRN2", target_bir_lowering=False, debug=False)
    dt_r1 = R1DT

    # ---------------- DRAM I/O ----------------
    xD = nc.dram_tensor("x", [GPC, N, D], IODT, kind="ExternalInput").ap()
    outD = nc.dram_tensor("out", [GPC, N, D], IODT, kind="ExternalOutput").ap()

    def cin(name, shape, dt=F32):
        return nc.dram_tensor(name, list(shape), dt, kind="ExternalInput").ap()

    embbdD = cin("embbd", [38, 2 * H])              # padded blockdiag emb_w
    b1nrD = cin("b1nr", [1, 2 * H])                 # node_b1 dup as row
    fb3rD = cin("fb3r", [1, 3])                     # final_b[3:] row
    embbD = cin("embb", [2 * H, 1])                 # emb_b dup
    AbdD = cin("Abd", [2 * H, 2 * H])               # blockdiag edge_w1[:H]
    BbdD = cin("Bbd", [2 * H, 2 * H])               # blockdiag edge_w1[H:2H]
    w1rmD = cin("w1rm", [2, 2 * H], ZDT)            # masked w1r rows
    b1cD = cin("b1c", [2 * H, 1])                   # edge_b1 dup
    maskbdD = cin("maskbd", [2, 2 * H])             # graph masks
    W2C1bdD = cin("W2C1bd", [2 * H, 2 * H], dt_r1)  # blockdiag edge_w2@coord_w1
    b2c1cD = cin("b2c1c", [2 * H, 1])               # (b2@C1 + c1b) dup
    c2scD = cin("c2sc", [2 * H, 16 * NCH], dt_r1)   # shifted coord_w2 cols
    W2bdD = cin("W2bd", [2 * H, 2 * H], dt_r1)      # blockdiag edge_w2
    b2x64D = cin("b2x64", [2 * H, 1])               # 64*edge_b2 dup
    W1nh2D = cin("W1nh2", [2 * H, H])               # node_w1[:H] dup'd
    W1nm2D = cin("W1nm2", [2 * H, H])               # node_w1[H:] dup'd
    W2nbdD = cin("W2nbd", [2 * H, 2 * H])           # blockdiag node_w2
    b2ncD = cin("b2nc", [2 * H, 1])                 # node_b2 dup
    Wf3D = cin("Wf3", [2 * H, 3])                   # final_w[:, 3:] dup'd

    from contextlib import ExitStack

    with tile.TileContext(nc) as tc:
        with ExitStack() as ctx:
            statics = ctx.enter_context(tc.tile_pool(name="statics", bufs=1))
            pers = ctx.enter_context(tc.tile_pool(name="pers", bufs=1))
            sb2 = ctx.enter_context(tc.tile_pool(name="sb2", bufs=2))
            big = ctx.enter_context(tc.tile_pool(name="big", bufs=2))
            zp = ctx.enter_context(tc.tile_pool(name="zp", bufs=2, space="PSUM"))
            tp = ctx.enter_context(tc.tile_pool(name="tp", bufs=2, space="PSUM"))
            sp = ctx.enter_context(tc.tile_pool(name="sp", bufs=2, space="PSUM"))
            dsc = ctx.enter_context(tc.tile_pool(name="dsc", bufs=2,
                                                 space="DRAM"))
            # ---- load constants into SBUF once ----
            def ld(apD, dt=None):
                t = statics.tile(list(apD.shape), dt or apD.dtype,
                                 tag=f"c_{apD.name}")
                nc.sync.dma_start(out=t[:], in_=apD)
                return t

            embbd = ld(embbdD)
            b1nr = ld(b1nrD)
            fb3r = ld(fb3rD)
            embb = ld(embbD)

            # ---- generated constants (identity / delta patterns) ----
            I128 = statics.tile([2 * H, 2 * H], F32, tag="c_I128")
            nc.vector.memset(I128[:], 1.0)
            nc.gpsimd.affine_select(
                out=I128[:], in_=I128[:], pattern=[[-1, 2 * H]],
                compare_op=OP.is_equal, fill=0.0, channel_multiplier=1)
            onesr = statics.tile([1, 2 * H], F32, tag="c_onesr")
            nc.vector.memset(onesr[:], 1.0)
            # Tbig2[j, (jj ii)] = delta(j, jj)
            Tbig2 = statics.tile([N, NSQ], ZDT, tag="c_Tbig2")
            nc.vector.memset(Tbig2[:], 1.0)
            nc.gpsimd.affine_select(
                out=Tbig2[:], in_=Tbig2[:], pattern=[[-1, N], [0, N]],
                compare_op=OP.is_equal, fill=0.0, channel_multiplier=1)
            Abd = ld(AbdD)
            Bbd = ld(BbdD)
            b1c = ld(b1cD)
            maskbd = ld(maskbdD)
            W2C1bd = ld(W2C1bdD)
            b2c1c = ld(b2c1cD)
            c2sc = ld(c2scD)
            W2bd = ld(W2bdD)
            b2x64 = ld(b2x64D)
            W1nh2 = ld(W1nh2D)
            W1nm2 = ld(W1nm2D)
            W2nbd = ld(W2nbdD)
            b2nc = ld(b2ncD)
            Wf3 = ld(Wf3D)

            # persistent per-parity combo tiles
            cA_lhs = []  # [66, 128]: rows 0:64 HAstack (per pair), 64:66 w1r
            cA_rhs = []  # [66, 4096]: rows 0:64 Tbig static, 64:66 d2 flat
            wT = []      # [128, 128] blockdiag w target, zeroed once
            for par in range(2):
                lt = pers.tile([N + 2, 2 * H], ZDT, tag=f"cAl{par}")
                nc.sync.dma_start(out=lt[N:N + 2, :], in_=w1rmD)
                rt = pers.tile([N + 2, NSQ], ZDT, tag=f"cAr{par}")
                # Tbig[i, (jj ii)] = delta(i, ii)
                nc.vector.memset(rt[0:N, :], 1.0)
                nc.gpsimd.affine_select(
                    out=rt[0:N, :], in_=rt[0:N, :],
                    pattern=[[0, N], [-1, N]],
                    compare_op=OP.is_equal, fill=0.0, channel_multiplier=1)
                wt = pers.tile([2 * H, 2 * H], F32, tag=f"wT{par}")
                nc.vector.memset(wt[:], 0.0)
                cA_lhs.append(lt)
                cA_rhs.append(rt)
                wT.append(wt)

            for p in range(PAIRS):
                par = p % 2
                # ---- per-pair loads ----
                x_pair_h = sb2.tile([2 * N, D], IODT)
                nc.gpsimd.dma_start(
                    out=x_pair_h[:],
                    in_=xD[2 * p:2 * p + 2, :, :].rearrange(
                        "g i d -> (g i) d"),
                )
                if IO_F16:
                    x_pair = sb2.tile([2 * N, D], F32)
                    nc.vector.tensor_copy(out=x_pair[:], in_=x_pair_h[:])
                else:
                    x_pair = x_pair_h
                # x^T via PE transpose + DRAM bounce into padded per-graph
                # layout (graph 0 rows 0:6, graph 1 rows 32:38 -- PE-legal
                # partition bases)
                pxT = sp.tile([D, 2 * N], F32, tag="sp")
                nc.tensor.transpose(pxT[:], x_pair[:], I128[:])
                xT6 = sb2.tile([D, 2 * N], F32)
                nc.vector.tensor_copy(out=xT6[:], in_=pxT[:])
                xts = dsc.tile([D, 2 * N], F32, tag="xts")
                nc.gpsimd.dma_start(out=xts[:], in_=xT6[:])
                xT12p = sb2.tile([38, N], F32)
                nc.vector.memset(xT12p[:], 0.0)
                for gg in range(2):
                    nc.gpsimd.dma_start(out=xT12p[32 * gg:32 * gg + D, :],
                                        in_=xts[:, gg * N:(gg + 1) * N])

                # ---- embedding (transposed pair layout) ----
                ph = sp.tile([2 * H, N], F32, tag="sp")
                nc.tensor.matmul(ph[:], _mm(embbd[:]), _mm(xT12p[:]),
                                 start=True, stop=True)
                hT2 = sb2.tile([2 * H, N], F32)
                nc.vector.tensor_scalar_add(hT2[:], ph[:], embb[:, 0:1])

                # ---- HA/HB stacks ----
                pHA = sp.tile([N, 2 * H], F32, tag="sp")
                nc.tensor.matmul(pHA[:], _mm(hT2[:]), _mm(Abd[:]),
                                 start=True, stop=True)
                nc.vector.tensor_copy(out=cA_lhs[par][0:N, :], in_=pHA[:])
                pHB = sp.tile([N, 2 * H], F32, tag="sp")
                nc.tensor.matmul(pHB[:], _mm(hT2[:]), _mm(Bbd[:]),
                                 start=True, stop=True)
                HBst = sb2.tile([N, 2 * H], ZDT)
                nc.scalar.copy(out=HBst[:], in_=pHB[:])

                # ---- pairwise distances d2 ----
                pm2c = sb2.tile([38, N], F32)
                nc.vector.tensor_scalar_mul(pm2c[:], xT12p[:], -2.0)
                posTbd = sb2.tile([38, 2 * N], F32)
                nc.vector.memset(posTbd[:], 0.0)
                for gg in range(2):
                    nc.vector.tensor_copy(
                        out=posTbd[32 * gg:32 * gg + 3,
                                   gg * N:(gg + 1) * N],
                        in_=xT12p[32 * gg:32 * gg + 3, :])
                n2c = sb2.tile([2 * N, 1], F32)
                sq3 = sb2.tile([2 * N, 3], F32)
                nc.vector.scalar_tensor_tensor(
                    out=sq3[:], in0=x_pair[:, 0:3], scalar=1.0,
                    in1=x_pair[:, 0:3], op0=OP.mult, op1=OP.mult,
                    accum_out=n2c[:, 0:1])
                n2s = dsc.tile([2 * N, 1], F32, tag="n2s")
                nc.gpsimd.dma_start(out=n2s[:], in_=n2c[:])
                n2r = sb2.tile([2, N], F32)
                nc.gpsimd.dma_start(
                    out=n2r[:],
                    in_=n2s[:].rearrange("(g j) o -> g (j o)", g=2))
                pg = sp.tile([2 * H, N], F32, tag="sp")
                nc.tensor.matmul(pg[:], _mm(posTbd[:]), _mm(pm2c[:]),
                                 start=True, stop=False)
                nc.tensor.matmul(pg[:], _mm(maskbd[:]), _mm(n2r[:]),
                                 start=False, stop=True)
                d2 = sb2.tile([2 * N, N], ZDT)
                nc.vector.tensor_scalar_add(d2[:], pg[:], n2c[:, 0:1])
                # flatten [(g j), i] -> combo rows [g, (j i)] via DRAM
                # bounce; valid because d2 is symmetric
                d2s = dsc.tile([2 * N, N], ZDT, tag="d2s")
                nc.gpsimd.dma_start(out=d2s[:], in_=d2[:])
                nc.gpsimd.dma_start(
                    out=cA_rhs[par][N:N + 2, :],
                    in_=d2s[:].rearrange("(g j) i -> g (j i)", g=2))

                # ---- big phase: z1 -> r1 -> t -> u -> w ----
                r1 = big.tile([2 * H, NSQ], dt_r1, tag="r1")
                u = big.tile([2 * H, NSQ], dt_r1, tag="u")
                pw = sp.tile([2 * NCH, CH], F32, tag="sp")
                for cb in range(NCH):
                    pz = zp.tile([2 * H, CH], F32, tag="z")
                    nc.tensor.matmul(
                        pz[:], _mm(cA_lhs[par][:]),
                        _mm(cA_rhs[par][:, cb * CH:(cb + 1) * CH]),
                        start=True, stop=False)
                    nc.tensor.matmul(
                        pz[:], _mm(HBst[:]),
                        _mm(Tbig2[:, cb * CH:(cb + 1) * CH]),
                        start=False, stop=True)
                    r1_sl = r1[:, cb * CH:(cb + 1) * CH]
                    if cb % 2 == 0:
                        nc.scalar.activation(out=r1_sl, in_=pz[:], func=AF.Relu,
                                             bias=b1c[:, 0:1], scale=1.0)
                    else:
                        nc.vector.tensor_scalar(
                            out=r1_sl, in0=pz[:], scalar1=b1c[:, 0:1],
                            scalar2=0.0, op0=OP.add, op1=OP.max)
                    pt = tp.tile([2 * H, CH], F32, tag="t")
                    nc.tensor.matmul(pt[:], W2C1bd[:].bitcast(_mmdt(R1DT)),
                                     _mm(r1_sl), start=True, stop=True)
                    u_sl = u[:, cb * CH:(cb + 1) * CH]
                    if cb % 2 == 1:
                        nc.scalar.activation(out=u_sl, in_=pt[:], func=AF.Relu,
                                             bias=b2c1c[:, 0:1], scale=1.0)
                    else:
                        nc.vector.tensor_scalar(
                            out=u_sl, in0=pt[:], scalar1=b2c1c[:, 0:1],
                            scalar2=0.0, op0=OP.add, op1=OP.max)
                    nc.tensor.matmul(pw[:],
                                     c2sc[:, 16 * cb:16 * (cb + 1)]
                                     .bitcast(_mmdt(R1DT)), _mm(u_sl),
                                     start=(cb == 0), stop=(cb == NCH - 1),
                                     skip_group_check=True)

                # ---- w -> blockdiag via SBUF + DRAM bounce ----
                w16 = sb2.tile([2 * NCH, CH], F32)
                nc.scalar.copy(out=w16[:], in_=pw[:])
                ws = dsc.tile([2 * NCH, CH], F32, tag="ws")
                nc.gpsimd.dma_start(out=ws[:], in_=w16[:])
                for gg in range(2):
                    # DRAM-side gather: row 2cb+gg, chunk-local (jl, i)
                    nc.gpsimd.dma_start(
                        out=wT[par][gg * N:(gg + 1) * N,
                                    gg * N:(gg + 1) * N],
                        in_=ws[:].rearrange("(cb g) (jl i) -> g cb jl i",
                                            g=2, jl=JB)[gg],
                    )

                # ---- R = sum_j r1 (log-tree fold over outer j) ----
                tr = big.tile([2 * H, NSQ // 2], dt_r1, tag="tr")
                nc.vector.tensor_add(tr[:], r1[:, 0:NSQ // 2],
                                     r1[:, NSQ // 2:NSQ])
                wdt = NSQ // 4
                while wdt >= N:
                    nc.vector.tensor_add(tr[:, 0:wdt], tr[:, 0:wdt],
                                         tr[:, wdt:2 * wdt])
                    wdt //= 2
                # ---- m = R @ W2 + 64*b2 (T layout) ----
                pm = sp.tile([2 * H, N], F32, tag="sp")
                nc.tensor.matmul(pm[:], W2bd[:].bitcast(_mmdt(R1DT)),
                                 _mm(tr[:, 0:N]), start=True, stop=True)
                mTs = sb2.tile([2 * H, N], F32)
                nc.vector.tensor_scalar_add(mTs[:], pm[:], b2x64[:, 0:1])

                # ---- node MLP ----
                # init-mm seeds the full bank with the bias so the per-graph
                # matmuls can all accumulate (single start per bank region)
                pq = sp.tile([2 * H, N], F32, tag="sp")
                nc.tensor.matmul(pq[:], _mm(b1nr[:]), _mm(onesr[:, 0:N]),
                                 start=True, stop=False)
                for gg in range(2):
                    o = pq[gg * H:(gg + 1) * H, :]
                    sl = slice(gg * H, (gg + 1) * H)
                    nc.tensor.matmul(o, _mm(W1nh2[sl, :]), _mm(hT2[sl, :]),
                                     start=False, stop=False)
                    nc.tensor.matmul(o, _mm(W1nm2[sl, :]), _mm(mTs[sl, :]),
                                     start=False, stop=(gg == 1))
                qT = sb2.tile([2 * H, N], F32)
                nc.scalar.activation(out=qT[:], in_=pq[:], func=AF.Relu,
                                     bias=0.0, scale=1.0)
                pn2 = sp.tile([2 * H, N], F32, tag="sp")
                nc.tensor.matmul(pn2[:], _mm(W2nbd[:]), _mm(qT[:]),
                                 start=True, stop=True)
                hp = sb2.tile([2 * H, N], F32)
                nc.vector.scalar_tensor_tensor(
                    out=hp[:], in0=pn2[:], scalar=b2nc[:, 0:1], in1=hT2[:],
                    op0=OP.add, op1=OP.add)

                # ---- coordinate aggregation ----
                pa = sb2.tile([2 * N, 4], F32)
                nc.vector.tensor_copy(out=pa[:, 0:3], in_=x_pair[:, 0:3])
                nc.vector.memset(pa[:, 3:4], 1.0)
                pswp = sp.tile([2 * N, 4], F32, tag="sp")
                nc.tensor.matmul(pswp[:], _mm(wT[par][:]), _mm(pa[:]),
                                 start=True, stop=True)

                # ---- velocity head ----
                pv = sp.tile([2 * N, 3], F32, tag="sp")
                nc.tensor.matmul(pv[:], _mm(onesr[:]), _mm(fb3r[:]),
                                 start=True, stop=False)
                for gg in range(2):
                    sl = slice(gg * H, (gg + 1) * H)
                    nc.tensor.matmul(pv[gg * N:(gg + 1) * N, :],
                                     _mm(hp[sl, :]), _mm(Wf3[sl, :]),
                                     start=False, stop=(gg == 1))

                # ---- assemble output ----
                op_t = sb2.tile([2 * N, D], IODT)
                tmp3 = sb2.tile([2 * N, 3], F32)
                nc.vector.scalar_tensor_tensor(
                    out=tmp3[:], in0=x_pair[:, 0:3], scalar=pswp[:, 3:4],
                    in1=pswp[:, 0:3], op0=OP.mult, op1=OP.subtract)
                nc.vector.scalar_tensor_tensor(
                    out=op_t[:, 0:3], in0=tmp3[:], scalar=1.0 / N,
                    in1=x_pair[:, 0:3], op0=OP.mult, op1=OP.add)
                nc.vector.tensor_add(op_t[:, 3:6], pv[:], x_pair[:, 3:6])
                nc.gpsimd.dma_start(
                    out=outD[2 * p:2 * p + 2, :, :].rearrange(
                        "g i d -> (g i) d"),
                    in_=op_t[:])

    nc.compile()
    return nc


def make_consts(emb_w, emb_b, edge_w1, edge_b1, edge_w2, edge_b2,
                node_w1, node_b1, node_w2, node_b2,
                coord_w1, coord_b1, coord_w2, final_w, final_b):
    f = np.float32
    E = np.asarray(emb_w, f)
    A = np.asarray(edge_w1[0:H], f)
    Bm = np.asarray(edge_w1[H:2 * H], f)
    w1r = np.asarray(edge_w1[2 * H], f)
    W2 = np.asarray(edge_w2, f)
    C1 = np.asarray(coord_w1, f)
    W2C1 = (W2 @ C1).astype(f)
    b2c1 = (np.asarray(edge_b2, f) @ C1 + np.asarray(coord_b1, f)).astype(f)
    c2 = np.asarray(coord_w2, f)[:, 0]

    def bd(M):
        out = np.zeros((2 * M.shape[0], 2 * M.shape[1]), f)
        out[:M.shape[0], :M.shape[1]] = M
        out[M.shape[0]:, M.shape[1]:] = M
        return out

    def dup(v):
        return np.tile(np.asarray(v, f), 2)[:, None].astype(f)

    w1rm = np.zeros((2, 2 * H), f)
    w1rm[0, 0:H] = w1r
    w1rm[1, H:2 * H] = w1r
    # c2sc: for chunk cb, lhsT = c2sc[:, 16cb:16cb+16]; column m of that
    # slice carries c2 masked to graph gg iff m == 2*cb+gg, so the 8
    # accumulating matmuls scatter chunk cb's w into psum rows 2cb:2cb+2.
    c2sc = np.zeros((2 * H, 16 * NCH), f)
    for cb in range(NCH):
        for gg in range(2):
            c2sc[gg * H:(gg + 1) * H, 16 * cb + 2 * cb + gg] = c2
    maskbd = np.zeros((2, 2 * H), f)
    maskbd[0, 0:N] = 1.0
    maskbd[1, N:2 * N] = 1.0
    r1dt = np.dtype("bfloat16") if R1_BF16 else f
    zdt = np.dtype("bfloat16") if Z_BF16 else f
    W1 = np.asarray(node_w1, f)
    embbd38 = np.zeros((38, 2 * H), f)
    embbd38[0:D, 0:H] = E
    embbd38[32:32 + D, H:2 * H] = E
    consts = {
        "embbd": embbd38,
        "b1nr": np.tile(np.asarray(node_b1, f), 2)[None, :],
        "fb3r": np.asarray(final_b, f)[None, 3:6],
        "embb": dup(emb_b),
        "Abd": bd(A),
        "Bbd": bd(Bm),
        "w1rm": w1rm.astype(zdt),
        "b1c": dup(edge_b1),
        "maskbd": maskbd,
        "W2C1bd": bd(W2C1).astype(r1dt),
        "b2c1c": dup(b2c1),
        "c2sc": c2sc.astype(r1dt),
        "W2bd": bd(W2).astype(r1dt),
        "b2x64": dup(np.asarray(edge_b2, f) * N),
        "W1nh2": np.concatenate([W1[0:H], W1[0:H]], 0),
        "W1nm2": np.concatenate([W1[H:2 * H], W1[H:2 * H]], 0),
        "W2nbd": bd(np.asarray(node_w2, f)),
        "b2nc": dup(node_b2),
        "Wf3": np.tile(np.asarray(final_w, f)[:, 3:6], (2, 1)),
    }
    return consts


# ---------------------------------------------------------------------------
# Runtime: cached jitted SPMD executable + device-resident constants.
# ---------------------------------------------------------------------------

_CACHE = {}


class _Runtime:
    def __init__(self):
        import jax
        from jax.sharding import Mesh, PartitionSpec, NamedSharding
        try:
            from jax import shard_map as _shard_map

            def shard_map(f, mesh, in_specs, out_specs, check_rep):
                return _shard_map(f, mesh=mesh, in_specs=in_specs,
                                  out_specs=out_specs, check_vma=check_rep)
        except ImportError:
            from jax.experimental.shard_map import shard_map
        from concourse.bass2jax import (_bass_exec_p, install_neuronx_cc_hook,
                                        partition_id_tensor)

        self.jax = jax
        nc = build_program()
        self.nc = nc
        install_neuronx_cc_hook()

        partition_name = (nc.partition_id_tensor.name
                          if nc.partition_id_tensor else None)
        assert nc.dbg_addr is None

        in_names, out_names, out_avals = [], [], []
        for alloc in nc.m.functions[0].allocations:
            if not isinstance(alloc, mybir.MemoryLocationSet):
                continue
            name = alloc.memorylocations[0].name
            if alloc.kind == "ExternalInput":
                if name != partition_name:
                    in_names.append(name)
            elif alloc.kind == "ExternalOutput":
                out_names.append(name)
                out_avals.append(jax.core.ShapedArray(
                    tuple(alloc.tensor_shape), mybir.dt.np(alloc.dtype)))
        self.in_names = in_names
        self.out_names = out_names
        in_names_full = tuple(in_names + out_names +
                              ([partition_name] if partition_name else []))
        out_avals_t = tuple(out_avals)

        def _body(*args):
            operands = list(args)
            if partition_name is not None:
                operands.append(partition_id_tensor())
            outs = _bass_exec_p.bind(
                *operands, out_avals=out_avals_t, in_names=in_names_full,
                out_names=tuple(out_names),
                lowering_input_output_aliases=(),
                sim_require_finite=True, sim_require_nnan=True, nc=nc)
            return tuple(outs)

        devices = jax.devices()[:NCORES]
        assert len(devices) == NCORES
        mesh = Mesh(np.asarray(devices), ("core",))
        self.sharding = NamedSharding(mesh, PartitionSpec("core"))
        n_args = len(in_names) + len(out_names)
        self.fn = jax.jit(
            shard_map(_body, mesh=mesh,
                      in_specs=(PartitionSpec("core"),) * n_args,
                      out_specs=(PartitionSpec("core"),) * len(out_names),
                      check_rep=False),
            keep_unused=True)

        # dead output-shaped buffers (the exec lowering allocates fresh
        # result buffers; these operands are never read)
        self.zero_outs = [
            jax.device_put(
                np.zeros((NCORES * a.shape[0], *a.shape[1:]), a.dtype),
                self.sharding)
            for a in out_avals]
        self.const_digest = None
        self.dev_consts = None

    def upload_consts(self, consts, digest):
        jax = self.jax
        names = [n for n in self.in_names if n != "x"]
        globs = []
        for n in names:
            c = np.ascontiguousarray(consts[n])
            g = np.broadcast_to(c[None], (NCORES,) + c.shape)
            globs.append(np.ascontiguousarray(
                g.reshape(NCORES * c.shape[0], *c.shape[1:])))
        devs = jax.device_put(globs, [self.sharding] * len(globs))
        jax.block_until_ready(devs)
        self.dev_consts = dict(zip(names, devs))
        self.const_digest = digest

    def __call__(self, x16, consts, digest):
        if digest != self.const_digest:
            self.upload_consts(consts, digest)
        args = [x16 if n == "x" else self.dev_consts[n]
                for n in self.in_names] + self.zero_outs
        outs = self.fn(*args)
        return np.asarray(outs[0])


def _get_runtime():
    if "rt" not in _CACHE:
        _CACHE["rt"] = _Runtime()
    return _CACHE["rt"]


def kernel(x, emb_w, emb_b, edge_w1, edge_b1, edge_w2, edge_b2,
           node_w1, node_b1, node_w2, node_b2,
           coord_w1, coord_b1, coord_w2, final_w, final_b):
    weights = dict(
        emb_w=emb_w, emb_b=emb_b, edge_w1=edge_w1, edge_b1=edge_b1,
        edge_w2=edge_w2, edge_b2=edge_b2, node_w1=node_w1, node_b1=node_b1,
        node_w2=node_w2, node_b2=node_b2, coord_w1=coord_w1,
        coord_b1=coord_b1, coord_w2=coord_w2, final_w=final_w,
        final_b=final_b)
    rt = _get_runtime()

    h = hashlib.blake2b(digest_size=16)
    for name in WEIGHT_NAMES:
        h.update(np.ascontiguousarray(
            np.asarray(weights[name], np.float32)).tobytes())
    digest = h.digest()

    consts = None
    if digest != rt.const_digest:
        consts = make_consts(**weights)

    x16 = np.ascontiguousarray(
        np.asarray(x, np.float32).reshape(NCORES * GPC, N, D)).astype(IODT_NP)
    out = rt(x16, consts, digest)
    return out.reshape(B, S, N, D).astype(np.float32)


# revision 4
# speedup vs baseline: 18.4420x; 1.0624x over previous
"""EquivariantGNN layer on 8 Trainium2 NeuronCores.

Data-parallel over the 256 graphs (32 graphs/core, processed as 16
pairs packed on 128 partitions). Per pair, the N^2 edge work lives in a
[128, 4096] "transposed" layout: partitions = (graph, channel), free =
(j, i) with i innermost.

Algebraic restructuring vs the reference:
  - edge MLP layer 1 is separable: e_in @ W1 = HA[i] + HB[j] + w1r*d2
    -> built by PE matmuls straight into PSUM (identity-pattern rhs),
    b1 folded into the relu bias.
  - coord MLP layer 1 is fused into edge layer 2: t = r1 @ (W2@C1),
    so m_ij is never materialized; m_i = (sum_j r1) @ W2.
  - pos aggregation via sum_j w_ij * [pos_j|1] = one K=128 matmul after
    un-flattening w into a block-diagonal [128,128] tile by DMA.

Host/runtime restructuring vs the v0 session:
  - x and out cross the (slow, ~57ms RTT / ~25MB/s) axon tunnel in
    fp16, halving activation transfer bytes.
  - the jitted SPMD executable is cached across kernel() calls
    (rebuilding it cost ~0.6s/call in re-trace + BIR re-verify).
  - weight-derived constant tensors are uploaded once and kept
    device-resident across calls (keyed by a content digest); only x
    moves per call.
  - big-phase matmuls run as float32r (4x PE rate at N>=512) and the
    r1/u edge intermediates are bf16.
"""

import hashlib
import sys

sys.path.insert(0, "/opt/trn_rl_repo")

import numpy as np

import concourse.bass as bass  # noqa: F401  (registers AP machinery)
import concourse.tile as tile
from concourse import bacc, mybir

N = 64          # nodes per graph
H = 64          # hidden
D = 6           # in/out feature dim
B, S = 8, 32
G = B * S       # 256 graphs
NCORES = 8
GPC = G // NCORES   # 32 graphs per core
PAIRS = GPC // 2    # 16 pairs
NSQ = N * N         # 4096
CH = 512            # free-dim chunk (one PSUM bank of f32)
NCH = NSQ // CH     # 8 chunks per pair
JB = CH // N        # 8 j-values per chunk

F32 = mybir.dt.float32
F32R = mybir.dt.float32r
BF16 = mybir.dt.bfloat16
F16 = mybir.dt.float16

# ---- perf knobs ----
MM_F32R = False     # float32r needs explicit rounding chains; fp16 instead
Z_F16 = True        # z-matmul operands (HA/HB/d2/patterns) in fp16 (1 cyc/row)
R1_F16 = True       # r1/u intermediates in fp16 (2x DVE, ~1e-3 rounding)
IO_F16 = True       # x/out cross the axon tunnel as fp16

R1DT = F16 if R1_F16 else F32
ZDT = F16 if Z_F16 else F32
IODT = F16 if IO_F16 else F32
IODT_NP = np.float16 if IO_F16 else np.float32
AF = mybir.ActivationFunctionType
OP = mybir.AluOpType

WEIGHT_NAMES = [
    "emb_w", "emb_b", "edge_w1", "edge_b1", "edge_w2", "edge_b2",
    "node_w1", "node_b1", "node_w2", "node_b2",
    "coord_w1", "coord_b1", "coord_w2", "final_w", "final_b",
]


def _mm(ap):
    """Matmul-operand view: float32 -> float32r when enabled."""
    if MM_F32R and ap.dtype == F32:
        return ap.bitcast(F32R)
    return ap


def _mmdt(dt):
    if MM_F32R and dt == F32:
        return F32R
    return dt


def build_program():
    nc = bacc.Bacc("TRN2", target_bir_lowering=False, debug=False)
    dt_r1 = R1DT

    # ---------------- DRAM I/O ----------------
    xD = nc.dram_tensor("x", [GPC, N, D], IODT, kind="ExternalInput").ap()
    outD = nc.dram_tensor("out", [GPC, N, D], IODT, kind="ExternalOutput").ap()

    def cin(name, shape, dt=F32):
        return nc.dram_tensor(name, list(shape), dt, kind="ExternalInput").ap()

    embbdD = cin("embbd", [38, 2 * H])              # padded blockdiag emb_w
    b1nrD = cin("b1nr", [1, 2 * H])                 # node_b1 dup as row
    fb3rD = cin("fb3r", [1, 3])                     # final_b[3:] row
    embbD = cin("embb", [2 * H, 1])                 # emb_b dup
    AbdD = cin("Abd", [2 * H, 2 * H])               # blockdiag edge_w1[:H]
    BbdD = cin("Bbd", [2 * H, 2 * H])               # blockdiag edge_w1[H:2H]
    w1rmD = cin("w1rm", [2, 2 * H], ZDT)            # masked w1r rows
    b1cD = cin("b1c", [2 * H, 1])                   # edge_b1 dup
    maskbdD = cin("maskbd", [2, 2 * H])             # graph masks
    W2C1bdD = cin("W2C1bd", [2 * H, 2 * H], dt_r1)  # blockdiag edge_w2@coord_w1
    b2c1cD = cin("b2c1c", [2 * H, 1])               # (b2@C1 + c1b) dup
    c2scD = cin("c2sc", [2 * H, 16 * NCH], dt_r1)   # shifted coord_w2 cols
    W2bdD = cin("W2bd", [2 * H, 2 * H], dt_r1)      # blockdiag edge_w2
    b2x64D = cin("b2x64", [2 * H, 1])               # 64*edge_b2 dup
    W1nh2D = cin("W1nh2", [2 * H, H])               # node_w1[:H] dup'd
    W1nm2D = cin("W1nm2", [2 * H, H])               # node_w1[H:] dup'd
    W2nbdD = cin("W2nbd", [2 * H, 2 * H])           # blockdiag node_w2
    b2ncD = cin("b2nc", [2 * H, 1])                 # node_b2 dup
    Wf3D = cin("Wf3", [2 * H, 3])                   # final_w[:, 3:] dup'd

    from contextlib import ExitStack

    with tile.TileContext(nc) as tc:
        with ExitStack() as ctx:
            statics = ctx.enter_context(tc.tile_pool(name="statics", bufs=1))
            pers = ctx.enter_context(tc.tile_pool(name="pers", bufs=1))
            sb2 = ctx.enter_context(tc.tile_pool(name="sb2", bufs=2))
            big = ctx.enter_context(tc.tile_pool(name="big", bufs=2))
            zp = ctx.enter_context(tc.tile_pool(name="zp", bufs=2, space="PSUM"))
            tp = ctx.enter_context(tc.tile_pool(name="tp", bufs=2, space="PSUM"))
            sp = ctx.enter_context(tc.tile_pool(name="sp", bufs=2, space="PSUM"))
            dsc = ctx.enter_context(tc.tile_pool(name="dsc", bufs=2,
                                                 space="DRAM"))
            # ---- load constants into SBUF once ----
            def ld(apD, dt=None):
                t = statics.tile(list(apD.shape), dt or apD.dtype,
                                 tag=f"c_{apD.name}")
                nc.sync.dma_start(out=t[:], in_=apD)
                return t

            embbd = ld(embbdD)
            b1nr = ld(b1nrD)
            fb3r = ld(fb3rD)
            embb = ld(embbD)

            # ---- generated constants (identity / delta patterns) ----
            I128 = statics.tile([2 * H, 2 * H], F32, tag="c_I128")
            nc.vector.memset(I128[:], 1.0)
            nc.gpsimd.affine_select(
                out=I128[:], in_=I128[:], pattern=[[-1, 2 * H]],
                compare_op=OP.is_equal, fill=0.0, channel_multiplier=1)
            onesr = statics.tile([1, 2 * H], F32, tag="c_onesr")
            nc.vector.memset(onesr[:], 1.0)
            # Tbig2[j, (jj ii)] = delta(j, jj)
            Tbig2 = statics.tile([N, NSQ], ZDT, tag="c_Tbig2")
            nc.vector.memset(Tbig2[:], 1.0)
            nc.gpsimd.affine_select(
                out=Tbig2[:], in_=Tbig2[:], pattern=[[-1, N], [0, N]],
                compare_op=OP.is_equal, fill=0.0, channel_multiplier=1)
            Abd = ld(AbdD)
            Bbd = ld(BbdD)
            b1c = ld(b1cD)
            maskbd = ld(maskbdD)
            W2C1bd = ld(W2C1bdD)
            b2c1c = ld(b2c1cD)
            c2sc = ld(c2scD)
            W2bd = ld(W2bdD)
            b2x64 = ld(b2x64D)
            W1nh2 = ld(W1nh2D)
            W1nm2 = ld(W1nm2D)
            W2nbd = ld(W2nbdD)
            b2nc = ld(b2ncD)
            Wf3 = ld(Wf3D)

            # persistent per-parity combo tiles
            cA_lhs = []  # [66, 128]: rows 0:64 HAstack (per pair), 64:66 w1r
            cA_rhs = []  # [66, 4096]: rows 0:64 Tbig static, 64:66 d2 flat
            wT = []      # [128, 128] blockdiag w target, zeroed once
            for par in range(2):
                lt = pers.tile([N + 2, 2 * H], ZDT, tag=f"cAl{par}")
                nc.sync.dma_start(out=lt[N:N + 2, :], in_=w1rmD)
                rt = pers.tile([N + 2, NSQ], ZDT, tag=f"cAr{par}")
                # Tbig[i, (jj ii)] = delta(i, ii)
                nc.vector.memset(rt[0:N, :], 1.0)
                nc.gpsimd.affine_select(
                    out=rt[0:N, :], in_=rt[0:N, :],
                    pattern=[[0, N], [-1, N]],
                    compare_op=OP.is_equal, fill=0.0, channel_multiplier=1)
                wt = pers.tile([2 * H, 2 * H], F32, tag=f"wT{par}")
                nc.vector.memset(wt[:], 0.0)
                cA_lhs.append(lt)
                cA_rhs.append(rt)
                wT.append(wt)

            for p in range(PAIRS):
                par = p % 2
                # ---- per-pair loads ----
                x_pair_h = sb2.tile([2 * N, D], IODT)
                nc.gpsimd.dma_start(
                    out=x_pair_h[:],
                    in_=xD[2 * p:2 * p + 2, :, :].rearrange(
                        "g i d -> (g i) d"),
                )
                if IO_F16:
                    x_pair = sb2.tile([2 * N, D], F32)
                    nc.vector.tensor_copy(out=x_pair[:], in_=x_pair_h[:])
                else:
                    x_pair = x_pair_h
                # x^T via PE transpose + DRAM bounce into padded per-graph
                # layout (graph 0 rows 0:6, graph 1 rows 32:38 -- PE-legal
                # partition bases)
                pxT = sp.tile([D, 2 * N], F32, tag="sp")
                nc.tensor.transpose(pxT[:], x_pair[:], I128[:])
                xT6 = sb2.tile([D, 2 * N], F32)
                nc.vector.tensor_copy(out=xT6[:], in_=pxT[:])
                xts = dsc.tile([D, 2 * N], F32, tag="xts")
                nc.gpsimd.dma_start(out=xts[:], in_=xT6[:])
                xT12p = sb2.tile([38, N], F32)
                nc.vector.memset(xT12p[:], 0.0)
                for gg in range(2):
                    nc.gpsimd.dma_start(out=xT12p[32 * gg:32 * gg + D, :],
                                        in_=xts[:, gg * N:(gg + 1) * N])

                # ---- embedding (transposed pair layout) ----
                ph = sp.tile([2 * H, N], F32, tag="sp")
                nc.tensor.matmul(ph[:], _mm(embbd[:]), _mm(xT12p[:]),
                                 start=True, stop=True)
                hT2 = sb2.tile([2 * H, N], F32)
                nc.vector.tensor_scalar_add(hT2[:], ph[:], embb[:, 0:1])

                # ---- HA/HB stacks ----
                pHA = sp.tile([N, 2 * H], F32, tag="sp")
                nc.tensor.matmul(pHA[:], _mm(hT2[:]), _mm(Abd[:]),
                                 start=True, stop=True)
                nc.vector.tensor_copy(out=cA_lhs[par][0:N, :], in_=pHA[:])
                pHB = sp.tile([N, 2 * H], F32, tag="sp")
                nc.tensor.matmul(pHB[:], _mm(hT2[:]), _mm(Bbd[:]),
                                 start=True, stop=True)
                HBst = sb2.tile([N, 2 * H], ZDT)
                nc.scalar.copy(out=HBst[:], in_=pHB[:])

                # ---- pairwise distances d2 ----
                pm2c = sb2.tile([38, N], F32)
                nc.vector.tensor_scalar_mul(pm2c[:], xT12p[:], -2.0)
                posTbd = sb2.tile([38, 2 * N], F32)
                nc.vector.memset(posTbd[:], 0.0)
                for gg in range(2):
                    nc.vector.tensor_copy(
                        out=posTbd[32 * gg:32 * gg + 3,
                                   gg * N:(gg + 1) * N],
                        in_=xT12p[32 * gg:32 * gg + 3, :])
                n2c = sb2.tile([2 * N, 1], F32)
                sq3 = sb2.tile([2 * N, 3], F32)
                nc.vector.scalar_tensor_tensor(
                    out=sq3[:], in0=x_pair[:, 0:3], scalar=1.0,
                    in1=x_pair[:, 0:3], op0=OP.mult, op1=OP.mult,
                    accum_out=n2c[:, 0:1])
                n2s = dsc.tile([2 * N, 1], F32, tag="n2s")
                nc.gpsimd.dma_start(out=n2s[:], in_=n2c[:])
                n2r = sb2.tile([2, N], F32)
                nc.gpsimd.dma_start(
                    out=n2r[:],
                    in_=n2s[:].rearrange("(g j) o -> g (j o)", g=2))
                pg = sp.tile([2 * H, N], F32, tag="sp")
                nc.tensor.matmul(pg[:], _mm(posTbd[:]), _mm(pm2c[:]),
                                 start=True, stop=False)
                nc.tensor.matmul(pg[:], _mm(maskbd[:]), _mm(n2r[:]),
                                 start=False, stop=True)
                d2 = sb2.tile([2 * N, N], ZDT)
                nc.vector.tensor_scalar_add(d2[:], pg[:], n2c[:, 0:1])
                # flatten [(g j), i] -> combo rows [g, (j i)] via DRAM
                # bounce; valid because d2 is symmetric
                d2s = dsc.tile([2 * N, N], ZDT, tag="d2s")
                nc.gpsimd.dma_start(out=d2s[:], in_=d2[:])
                nc.gpsimd.dma_start(
                    out=cA_rhs[par][N:N + 2, :],
                    in_=d2s[:].rearrange("(g j) i -> g (j i)", g=2))

                # ---- big phase: z1 -> r1 -> t -> u -> w ----
                r1 = big.tile([2 * H, NSQ], dt_r1, tag="r1")
                u = big.tile([2 * H, NSQ], dt_r1, tag="u")
                pw = sp.tile([2 * NCH, CH], F32, tag="sp")
                for cb in range(NCH):
                    pz = zp.tile([2 * H, CH], F32, tag="z")
                    nc.tensor.matmul(
                        pz[:], _mm(cA_lhs[par][:]),
                        _mm(cA_rhs[par][:, cb * CH:(cb + 1) * CH]),
                        start=True, stop=False)
                    nc.tensor.matmul(
                        pz[:], _mm(HBst[:]),
                        _mm(Tbig2[:, cb * CH:(cb + 1) * CH]),
                        start=False, stop=True)
                    r1_sl = r1[:, cb * CH:(cb + 1) * CH]
                    if cb % 2 == 0:
                        nc.scalar.activation(out=r1_sl, in_=pz[:], func=AF.Relu,
                                             bias=b1c[:, 0:1], scale=1.0)
                    else:
                        nc.vector.tensor_scalar(
                            out=r1_sl, in0=pz[:], scalar1=b1c[:, 0:1],
                            scalar2=0.0, op0=OP.add, op1=OP.max)
                    pt = tp.tile([2 * H, CH], F32, tag="t")
                    nc.tensor.matmul(pt[:], W2C1bd[:].bitcast(_mmdt(R1DT)),
                                     _mm(r1_sl), start=True, stop=True)
                    u_sl = u[:, cb * CH:(cb + 1) * CH]
                    if cb % 2 == 1:
                        nc.scalar.activation(out=u_sl, in_=pt[:], func=AF.Relu,
                                             bias=b2c1c[:, 0:1], scale=1.0)
                    else:
                        nc.vector.tensor_scalar(
                            out=u_sl, in0=pt[:], scalar1=b2c1c[:, 0:1],
                            scalar2=0.0, op0=OP.add, op1=OP.max)
                    nc.tensor.matmul(pw[:],
                                     c2sc[:, 16 * cb:16 * (cb + 1)]
                                     .bitcast(_mmdt(R1DT)), _mm(u_sl),
                                     start=(cb == 0), stop=(cb == NCH - 1),
                                     skip_group_check=True)

                # ---- w -> blockdiag via SBUF + DRAM bounce ----
                w16 = sb2.tile([2 * NCH, CH], F32)
                nc.scalar.copy(out=w16[:], in_=pw[:])
                ws = dsc.tile([2 * NCH, CH], F32, tag="ws")
                nc.gpsimd.dma_start(out=ws[:], in_=w16[:])
                for gg in range(2):
                    # DRAM-side gather: row 2cb+gg, chunk-local (jl, i)
                    nc.gpsimd.dma_start(
                        out=wT[par][gg * N:(gg + 1) * N,
                                    gg * N:(gg + 1) * N],
                        in_=ws[:].rearrange("(cb g) (jl i) -> g cb jl i",
                                            g=2, jl=JB)[gg],
                    )

                # ---- R = sum_j r1 (log-tree fold over outer j) ----
                tr = big.tile([2 * H, NSQ // 2], dt_r1, tag="tr")
                nc.vector.tensor_add(tr[:], r1[:, 0:NSQ // 2],
                                     r1[:, NSQ // 2:NSQ])
                wdt = NSQ // 4
                while wdt >= N:
                    nc.vector.tensor_add(tr[:, 0:wdt], tr[:, 0:wdt],
                                         tr[:, wdt:2 * wdt])
                    wdt //= 2
                # ---- m = R @ W2 + 64*b2 (T layout) ----
                pm = sp.tile([2 * H, N], F32, tag="sp")
                nc.tensor.matmul(pm[:], W2bd[:].bitcast(_mmdt(R1DT)),
                                 _mm(tr[:, 0:N]), start=True, stop=True)
                mTs = sb2.tile([2 * H, N], F32)
                nc.vector.tensor_scalar_add(mTs[:], pm[:], b2x64[:, 0:1])

                # ---- node MLP ----
                # init-mm seeds the full bank with the bias so the per-graph
                # matmuls can all accumulate (single start per bank region)
                pq = sp.tile([2 * H, N], F32, tag="sp")
                nc.tensor.matmul(pq[:], _mm(b1nr[:]), _mm(onesr[:, 0:N]),
                                 start=True, stop=False)
                for gg in range(2):
                    o = pq[gg * H:(gg + 1) * H, :]
                    sl = slice(gg * H, (gg + 1) * H)
                    nc.tensor.matmul(o, _mm(W1nh2[sl, :]), _mm(hT2[sl, :]),
                                     start=False, stop=False)
                    nc.tensor.matmul(o, _mm(W1nm2[sl, :]), _mm(mTs[sl, :]),
                                     start=False, stop=(gg == 1))
                qT = sb2.tile([2 * H, N], F32)
                nc.scalar.activation(out=qT[:], in_=pq[:], func=AF.Relu,
                                     bias=0.0, scale=1.0)
                pn2 = sp.tile([2 * H, N], F32, tag="sp")
                nc.tensor.matmul(pn2[:], _mm(W2nbd[:]), _mm(qT[:]),
                                 start=True, stop=True)
                hp = sb2.tile([2 * H, N], F32)
                nc.vector.scalar_tensor_tensor(
                    out=hp[:], in0=pn2[:], scalar=b2nc[:, 0:1], in1=hT2[:],
                    op0=OP.add, op1=OP.add)

                # ---- coordinate aggregation ----
                pa = sb2.tile([2 * N, 4], F32)
                nc.vector.tensor_copy(out=pa[:, 0:3], in_=x_pair[:, 0:3])
                nc.vector.memset(pa[:, 3:4], 1.0)
                pswp = sp.tile([2 * N, 4], F32, tag="sp")
                nc.tensor.matmul(pswp[:], _mm(wT[par][:]), _mm(pa[:]),
                                 start=True, stop=True)

                # ---- velocity head ----
                pv = sp.tile([2 * N, 3], F32, tag="sp")
                nc.tensor.matmul(pv[:], _mm(onesr[:]), _mm(fb3r[:]),
                                 start=True, stop=False)
                for gg in range(2):
                    sl = slice(gg * H, (gg + 1) * H)
                    nc.tensor.matmul(pv[gg * N:(gg + 1) * N, :],
                                     _mm(hp[sl, :]), _mm(Wf3[sl, :]),
                                     start=False, stop=(gg == 1))

                # ---- assemble output ----
                op_t = sb2.tile([2 * N, D], IODT)
                tmp3 = sb2.tile([2 * N, 3], F32)
                nc.vector.scalar_tensor_tensor(
                    out=tmp3[:], in0=x_pair[:, 0:3], scalar=pswp[:, 3:4],
                    in1=pswp[:, 0:3], op0=OP.mult, op1=OP.subtract)
                nc.vector.scalar_tensor_tensor(
                    out=op_t[:, 0:3], in0=tmp3[:], scalar=1.0 / N,
                    in1=x_pair[:, 0:3], op0=OP.mult, op1=OP.add)
                nc.vector.tensor_add(op_t[:, 3:6], pv[:], x_pair[:, 3:6])
                nc.gpsimd.dma_start(
                    out=outD[2 * p:2 * p + 2, :, :].rearrange(
                        "g i d -> (g i) d"),
                    in_=op_t[:])

    nc.compile()
    return nc


def make_consts(emb_w, emb_b, edge_w1, edge_b1, edge_w2, edge_b2,
                node_w1, node_b1, node_w2, node_b2,
                coord_w1, coord_b1, coord_w2, final_w, final_b):
    f = np.float32
    E = np.asarray(emb_w, f)
    A = np.asarray(edge_w1[0:H], f)
    Bm = np.asarray(edge_w1[H:2 * H], f)
    w1r = np.asarray(edge_w1[2 * H], f)
    W2 = np.asarray(edge_w2, f)
    C1 = np.asarray(coord_w1, f)
    W2C1 = (W2 @ C1).astype(f)
    b2c1 = (np.asarray(edge_b2, f) @ C1 + np.asarray(coord_b1, f)).astype(f)
    c2 = np.asarray(coord_w2, f)[:, 0]

    def bd(M):
        out = np.zeros((2 * M.shape[0], 2 * M.shape[1]), f)
        out[:M.shape[0], :M.shape[1]] = M
        out[M.shape[0]:, M.shape[1]:] = M
        return out

    def dup(v):
        return np.tile(np.asarray(v, f), 2)[:, None].astype(f)

    w1rm = np.zeros((2, 2 * H), f)
    w1rm[0, 0:H] = w1r
    w1rm[1, H:2 * H] = w1r
    # c2sc: for chunk cb, lhsT = c2sc[:, 16cb:16cb+16]; column m of that
    # slice carries c2 masked to graph gg iff m == 2*cb+gg, so the 8
    # accumulating matmuls scatter chunk cb's w into psum rows 2cb:2cb+2.
    c2sc = np.zeros((2 * H, 16 * NCH), f)
    for cb in range(NCH):
        for gg in range(2):
            c2sc[gg * H:(gg + 1) * H, 16 * cb + 2 * cb + gg] = c2
    maskbd = np.zeros((2, 2 * H), f)
    maskbd[0, 0:N] = 1.0
    maskbd[1, N:2 * N] = 1.0
    r1dt = np.float16 if R1_F16 else f
    zdt = np.float16 if Z_F16 else f
    W1 = np.asarray(node_w1, f)
    embbd38 = np.zeros((38, 2 * H), f)
    embbd38[0:D, 0:H] = E
    embbd38[32:32 + D, H:2 * H] = E
    consts = {
        "embbd": embbd38,
        "b1nr": np.tile(np.asarray(node_b1, f), 2)[None, :],
        "fb3r": np.asarray(final_b, f)[None, 3:6],
        "embb": dup(emb_b),
        "Abd": bd(A),
        "Bbd": bd(Bm),
        "w1rm": w1rm.astype(zdt),
        "b1c": dup(edge_b1),
        "maskbd": maskbd,
        "W2C1bd": bd(W2C1).astype(r1dt),
        "b2c1c": dup(b2c1),
        "c2sc": c2sc.astype(r1dt),
        "W2bd": bd(W2).astype(r1dt),
        "b2x64": dup(np.asarray(edge_b2, f) * N),
        "W1nh2": np.concatenate([W1[0:H], W1[0:H]], 0),
        "W1nm2": np.concatenate([W1[H:2 * H], W1[H:2 * H]], 0),
        "W2nbd": bd(np.asarray(node_w2, f)),
        "b2nc": dup(node_b2),
        "Wf3": np.tile(np.asarray(final_w, f)[:, 3:6], (2, 1)),
    }
    return consts


# ---------------------------------------------------------------------------
# Runtime: cached jitted SPMD executable + device-resident constants.
# ---------------------------------------------------------------------------

_CACHE = {}


class _Runtime:
    def __init__(self):
        import jax
        from jax.sharding import Mesh, PartitionSpec, NamedSharding
        try:
            from jax import shard_map as _shard_map

            def shard_map(f, mesh, in_specs, out_specs, check_rep):
                return _shard_map(f, mesh=mesh, in_specs=in_specs,
                                  out_specs=out_specs, check_vma=check_rep)
        except ImportError:
            from jax.experimental.shard_map import shard_map
        from concourse.bass2jax import (_bass_exec_p, install_neuronx_cc_hook,
                                        partition_id_tensor)

        self.jax = jax
        nc = build_program()
        self.nc = nc
        install_neuronx_cc_hook()

        partition_name = (nc.partition_id_tensor.name
                          if nc.partition_id_tensor else None)
        assert nc.dbg_addr is None

        in_names, out_names, out_avals = [], [], []
        for alloc in nc.m.functions[0].allocations:
            if not isinstance(alloc, mybir.MemoryLocationSet):
                continue
            name = alloc.memorylocations[0].name
            if alloc.kind == "ExternalInput":
                if name != partition_name:
                    in_names.append(name)
            elif alloc.kind == "ExternalOutput":
                out_names.append(name)
                out_avals.append(jax.core.ShapedArray(
                    tuple(alloc.tensor_shape), mybir.dt.np(alloc.dtype)))
        self.in_names = in_names
        self.out_names = out_names
        in_names_full = tuple(in_names + out_names +
                              ([partition_name] if partition_name else []))
        out_avals_t = tuple(out_avals)

        def _body(*args):
            operands = list(args)
            if partition_name is not None:
                operands.append(partition_id_tensor())
            outs = _bass_exec_p.bind(
                *operands, out_avals=out_avals_t, in_names=in_names_full,
                out_names=tuple(out_names),
                lowering_input_output_aliases=(),
                sim_require_finite=True, sim_require_nnan=True, nc=nc)
            return tuple(outs)

        devices = jax.devices()[:NCORES]
        assert len(devices) == NCORES
        mesh = Mesh(np.asarray(devices), ("core",))
        self.sharding = NamedSharding(mesh, PartitionSpec("core"))
        n_args = len(in_names) + len(out_names)
        self.fn = jax.jit(
            shard_map(_body, mesh=mesh,
                      in_specs=(PartitionSpec("core"),) * n_args,
                      out_specs=(PartitionSpec("core"),) * len(out_names),
                      check_rep=False),
            keep_unused=True)

        # dead output-shaped buffers (the exec lowering allocates fresh
        # result buffers; these operands are never read)
        self.zero_outs = [
            jax.device_put(
                np.zeros((NCORES * a.shape[0], *a.shape[1:]), a.dtype),
                self.sharding)
            for a in out_avals]
        self.const_digest = None
        self.dev_consts = None

    def upload_consts(self, consts, digest):
        jax = self.jax
        names = [n for n in self.in_names if n != "x"]
        globs = []
        for n in names:
            c = np.ascontiguousarray(consts[n])
            g = np.broadcast_to(c[None], (NCORES,) + c.shape)
            globs.append(np.ascontiguousarray(
                g.reshape(NCORES * c.shape[0], *c.shape[1:])))
        devs = jax.device_put(globs, [self.sharding] * len(globs))
        jax.block_until_ready(devs)
        self.dev_consts = dict(zip(names, devs))
        self.const_digest = digest

    def __call__(self, x16, consts, digest):
        if digest != self.const_digest:
            self.upload_consts(consts, digest)
        args = [x16 if n == "x" else self.dev_consts[n]
                for n in self.in_names] + self.zero_outs
        outs = self.fn(*args)
        return np.asarray(outs[0])


def _get_runtime():
    if "rt" not in _CACHE:
        _CACHE["rt"] = _Runtime()
    return _CACHE["rt"]


def kernel(x, emb_w, emb_b, edge_w1, edge_b1, edge_w2, edge_b2,
           node_w1, node_b1, node_w2, node_b2,
           coord_w1, coord_b1, coord_w2, final_w, final_b):
    weights = dict(
        emb_w=emb_w, emb_b=emb_b, edge_w1=edge_w1, edge_b1=edge_b1,
        edge_w2=edge_w2, edge_b2=edge_b2, node_w1=node_w1, node_b1=node_b1,
        node_w2=node_w2, node_b2=node_b2, coord_w1=coord_w1,
        coord_b1=coord_b1, coord_w2=coord_w2, final_w=final_w,
        final_b=final_b)
    rt = _get_runtime()

    h = hashlib.blake2b(digest_size=16)
    for name in WEIGHT_NAMES:
        h.update(np.ascontiguousarray(
            np.asarray(weights[name], np.float32)).tobytes())
    digest = h.digest()

    consts = None
    if digest != rt.const_digest:
        consts = make_consts(**weights)

    x16 = np.ascontiguousarray(
        np.asarray(x, np.float32).reshape(NCORES * GPC, N, D)).astype(IODT_NP)
    out = rt(x16, consts, digest)
    return out.reshape(B, S, N, D).astype(np.float32)
